# revision 11
# baseline (speedup 1.0000x reference)
"""GAT 2-layer kernel for 8 trn2 NeuronCores (SPMD, destination-sharded).

Host side: sorts edges by destination, groups them into per-core windows
(<=128 dest nodes, <=2048 edges each, A/B-alternating subtiles by source
range for int16 gather indices), builds all index/one-hot metadata.

Device side (per core, identical program):
  P1: hx = x @ [W1 | W1@a_l | W1@a_r] for all nodes -> fp16 gather tables
      (A/B row ranges); e_dst for the core's own nodes via x_local.
  P2: per window: dma_gather message rows, one-hot (DVE) aggregation
      matmuls into PSUM, e_dst expansion via host one-hotT matmuls,
      segment softmax (no max-subtraction; values are small), epilogue
      computes h2 rows = relu(out1) @ [W2@head_W | W2@a2_l | W2@a2_r].
  AllGather of h2 tables.
  P3: same edge machinery for layer 2; epilogue adds head bias; writes
      node-major output rows for the core's windows.
Host gathers per-core rows into the full [N, 64] output.
"""
import sys
import types

sys.path.insert(0, "/opt/trn_rl_repo")
import numpy as np

import time as _time
_T0 = _time.monotonic()
def _log(msg):
    print(f"[kernel +{_time.monotonic()-_T0:.0f}s] {msg}", flush=True)

import concourse.bass as bass
import concourse.mybir as mybir
import concourse.tile as tile
from concourse import bacc, bass_utils

# ---------------------------------------------------------------- problem dims
N_NODES = 50000
N_EDGES = 800000
IN_DIM = 128
HIDDEN = 32
NHEAD = 4
OUT_DIM = 64
NEG_SLOPE = 0.2
NCORES = 8
P = 128
SUBT = 16                 # subtiles per window (8 A-side + 8 B-side)
WCAP = SUBT * P           # edge slots per window
CALL = 1024               # dma_gather indices per call (= 8 subtiles)
NQ = 4                    # swdge queues to rotate
F16 = mybir.dt.float16
F32 = mybir.dt.float32
I16 = mybir.dt.int16

ROW1 = 256                # fp16 elems per L1 table row (512B): h(128) es(4) pad
ROW2 = 128                # fp16 elems per L2 table row (256B): m2(64) e2s(1) pad


# ================================================================ host prep
def _host_prep(x, edge_index, W1, attn1, W2, attn2, head_W, head_b):
    row = np.concatenate([edge_index[0], np.arange(N_NODES, dtype=np.int64)])
    col = np.concatenate([edge_index[1], np.arange(N_NODES, dtype=np.int64)])

    # sort edges by destination
    order = np.argsort(col, kind="stable")
    row = row[order].astype(np.int64)
    col = col[order].astype(np.int64)

    deg = np.bincount(col, minlength=N_NODES)          # per-dest edge count
    node_ptr = np.zeros(N_NODES + 1, dtype=np.int64)
    np.cumsum(deg, out=node_ptr[1:])

    bA = min(25088, (N_NODES // 2 // P) * P)            # fixed A/B source cut
    NTILE = (N_NODES + P - 1) // P                      # 391
    NPAD = NTILE * P                                    # 50048
    tA = bA // P
    tB = NTILE - tA
    assert tA * P <= 32767 and tB * P <= 32767

    # per-node side-A edge count
    degA = np.bincount(col[row < bA], minlength=N_NODES)
    degB = deg - degA

    # --- carve destination nodes into windows: span<=128, per-side<=8*128.
    # Force a window cut at node bA so core boundary 3|4 can align with it.
    windows = []
    n = 0
    CAP = 8 * P
    while n < N_NODES:
        n1 = n
        a = b = 0
        while n1 < N_NODES and n1 - n < P:
            if n1 == bA and n1 > n:
                break                                    # cut at bA
            da, db = degA[n1], degB[n1]
            if a + da > CAP or b + db > CAP:
                break
            a += da
            b += db
            n1 += 1
        if n1 == n:
            n1 = n + 1                                   # huge-degree node
        windows.append((n, n1))
        n = n1
    wA = next(i for i, (a, _) in enumerate(windows) if a >= bA)
    assert windows[wA][0] == bA

    wedges = np.array([node_ptr[b] - node_ptr[a] for a, b in windows])
    # cores 0-3 split windows [0, wA); cores 4-7 split [wA, end) (edge-balanced)
    def split4(wlist, elist):
        tgt = elist.sum() / 4
        out = [[] for _ in range(4)]
        acc = 0.0
        c = 0
        for i, we in zip(wlist, elist):
            if c < 3 and acc + we / 2 > tgt * (c + 1):
                c += 1
            out[c].append(i)
            acc += we
        return out

    lowW = list(range(wA))
    hiW = list(range(wA, len(windows)))
    core_w = (split4(lowW, wedges[lowW]) + split4(hiW, wedges[hiW]))
    W = max(len(ws) for ws in core_w)
    R2 = W * P
    assert 4 * R2 <= 32767, f"L2 table half too large: {4 * R2}"

    # ownership maps (global)
    owner = np.zeros(N_NODES, np.int64)
    localrow = np.zeros(N_NODES, np.int64)
    for c in range(NCORES):
        for wi, widx in enumerate(core_w[c]):
            wa, wb = windows[widx]
            owner[wa:wb] = c
            localrow[wa:wb] = wi * P + (np.arange(wa, wb) - wa)

    # --- per-core slot layout -------------------------------------------
    per_core = []
    for c in range(NCORES):
        ws = core_w[c]
        nw = len(ws)
        slots_row = np.zeros((W, SUBT, P), dtype=np.int64)   # global src node
        slots_col = np.full((W, SUBT, P), -1.0, dtype=np.float32)
        slots_valid = np.zeros((W, SUBT, P), dtype=bool)
        win_base = np.full(W, N_NODES, dtype=np.int64)
        for wi in range(nw):
            a, b = windows[ws[wi]]
            win_base[wi] = a
            e0, e1 = node_ptr[a], node_ptr[b]
            er = row[e0:e1]
            ec = col[e0:e1]
            sideA = er < bA
            for side in (0, 1):
                m = sideA if side == 0 else ~sideA
                rs, cs = er[m], ec[m]
                ns = rs.shape[0]
                assert ns <= CAP, f"side overflow {ns}"
                j = (np.arange(ns) // P) * 2 + side
                p = np.arange(ns) % P
                slots_row[wi, j, p] = rs
                slots_col[wi, j, p] = (cs - a).astype(np.float32)
                slots_valid[wi, j, p] = True
        per_core.append(dict(slots_row=slots_row, slots_col=slots_col,
                             slots_valid=slots_valid, win_base=win_base,
                             nw=nw))
    return dict(row=row, col=col, windows=windows, core_w=core_w, W=W,
                bA=bA, NTILE=NTILE, NPAD=NPAD, tA=tA, tB=tB,
                per_core=per_core, node_ptr=node_ptr, owner=owner,
                localrow=localrow)


def _wrap_idx(flat_i16):
    """[n] -> [128, n//16] wrapped-by-16 and replicated to 8 Q7 cores."""
    w = flat_i16.reshape(-1, 16).T.astype(np.int16)
    return np.tile(w, (8, 1))


def _build_core_inputs(pp, c, x, W1, attn1, W2, attn2, head_W, head_b):
    W = pp["W"]
    bA = pp["bA"]
    NPAD = pp["NPAD"]
    pc = pp["per_core"][c]
    slots_row = pc["slots_row"]            # [W, SUBT, P]
    slots_col = pc["slots_col"]
    win_base = pc["win_base"]

    # ---- weights (fused on host) ----
    a_l1 = attn1[:, :HIDDEN]               # [4, 32]
    a_r1 = attn1[:, HIDDEN:]
    Wl = np.zeros((IN_DIM, NHEAD), np.float32)
    Wr = np.zeros((IN_DIM, NHEAD), np.float32)
    W1r = W1.reshape(IN_DIM, NHEAD, HIDDEN)
    for h in range(NHEAD):
        Wl[:, h] = W1r[:, h, :] @ a_l1[h]
        Wr[:, h] = W1r[:, h, :] @ a_r1[h]
    w1ext = np.concatenate([W1, Wl, Wr], axis=1)         # [128, 136]

    a_l2 = attn2[0, :OUT_DIM]
    a_r2 = attn2[0, OUT_DIM:]
    w2ext = np.concatenate([W2 @ head_W,
                            (W2 @ a_l2)[:, None],
                            (W2 @ a_r2)[:, None]], axis=1)  # [128, 66]
    w2ext = np.pad(w2ext, ((0, 0), (0, 2)))                 # [128, 68]

    # ---- x transposed (global) and x local (window-ordered) ----
    xT = np.zeros((P, NPAD), np.float16)
    xT[:, :N_NODES] = x.T.astype(np.float16)
    xloc = np.zeros((W * P, IN_DIM), np.float16)
    for wi in range(W):
        a = win_base[wi]
        b = min(a + P, N_NODES)
        if a < N_NODES:
            xloc[wi * P:wi * P + (b - a)] = x[a:b].astype(np.float16)
    xlocT = np.ascontiguousarray(xloc.T)                   # [128, W*128]

    # ---- gather indices ----
    valid = pc["slots_valid"]
    rows = slots_row                                      # [W, SUBT, P]
    # L1: table A rows = node, table B rows = node - bA; pads -> 0
    l1_idx = np.zeros((W, SUBT, P), np.int64)
    l1_idx[:, 0::2, :] = rows[:, 0::2, :]
    l1_idx[:, 1::2, :] = rows[:, 1::2, :] - bA
    l1_idx[~valid] = 0
    assert l1_idx.min() >= 0 and l1_idx.max() <= 32767

    # L2: table rows = owner*W*128 + localrow; side A = ranks 0..3
    g2row = pp["owner"] * (W * P) + pp["localrow"]
    l2_idx = g2row[rows.reshape(-1)].reshape(W, SUBT, P).copy()
    l2_idx[:, 1::2, :] -= 4 * W * P
    l2_idx[~valid] = 0
    assert l2_idx.min() >= 0 and l2_idx.max() <= 32767, \
        f"{l2_idx.min()} {l2_idx.max()}"

    # per-call wrapped idx arrays: call k covers window wk side sk
    # call order: (w, side): idx list = slots [w, side::2, :] flattened (j,p)
    def call_list(idx3):
        calls = []
        for wi in range(W):
            for side in (0, 1):
                sl = idx3[wi, side::2, :].reshape(-1)      # [1024]
                calls.append(sl.astype(np.int16))
        return np.concatenate([_wrap_idx(s) for s in calls], axis=1)

    l1_idx_w = call_list(l1_idx)                           # [128, 2W*64]
    l2_idx_w = call_list(l2_idx)

    # ---- col_local values + one-hotT host matrices ----
    colv = np.ascontiguousarray(
        slots_col.transpose(2, 0, 1).reshape(P, W * SUBT)).astype(np.float16)
    # onehotT[s] [wnode, e]: ohT[w, j, q, p] = (slots_col[w, j, p] == q)
    ohT = np.zeros((W * SUBT * P, P), np.float16)
    sc = slots_col.reshape(W * SUBT, P)
    for s in range(W * SUBT):
        cl = sc[s]
        m = cl >= 0
        ohT[s * P + cl[m].astype(np.int64), np.nonzero(m)[0]] = 1.0

    iota16 = np.tile(np.arange(P, dtype=np.float16)[None, :], (P, 1))
    ident16 = np.eye(P, dtype=np.float16)
    headb = np.tile(head_b[None, :].astype(np.float32), (P, 1))

    return dict(
        xT=xT, xlocT=xlocT,
        w1ext=w1ext.astype(np.float16), w2ext=w2ext.astype(np.float16),
        l1idx=l1_idx_w.astype(np.int16), l2idx=l2_idx_w.astype(np.int16),
        colv=colv, ohT=ohT, iota=iota16, ident=ident16, headb=headb,
    )


# ================================================================ device build
def _build_program(W, tA, tB, NTILE):
    nc = bacc.Bacc("TRN2", target_bir_lowering=False, debug=False,
                   num_devices=NCORES, num_swdge_queues=NQ)
    NPAD = NTILE * P
    NW = W                      # windows per core
    NS = W * SUBT               # subtiles per core
    NCALL = 2 * W               # gather calls per layer (A+B per window)
    R2 = W * P                  # rows per rank in L2 table

    # ---------------- dram params
    d = {}
    d["xT"] = nc.dram_tensor("xT", [P, NPAD], F16, kind="ExternalInput").ap()
    d["xlocT"] = nc.dram_tensor("xlocT", [P, R2], F16,
                                kind="ExternalInput").ap()
    d["w1ext"] = nc.dram_tensor("w1ext", [P, 136], F16,
                                kind="ExternalInput").ap()
    d["w2ext"] = nc.dram_tensor("w2ext", [P, 68], F16,
                                kind="ExternalInput").ap()
    d["l1idx"] = nc.dram_tensor("l1idx", [P, NCALL * 64], I16,
                                kind="ExternalInput").ap()
    d["l2idx"] = nc.dram_tensor("l2idx", [P, NCALL * 64], I16,
                                kind="ExternalInput").ap()
    d["colv"] = nc.dram_tensor("colv", [P, NS], F16,
                               kind="ExternalInput").ap()
    d["ohT"] = nc.dram_tensor("ohT", [NS * P, P], F16,
                              kind="ExternalInput").ap()
    d["iota"] = nc.dram_tensor("iota", [P, P], F16, kind="ExternalInput").ap()
    d["ident"] = nc.dram_tensor("ident", [P, P], F16,
                                kind="ExternalInput").ap()
    d["headb"] = nc.dram_tensor("headb", [P, 64], F32,
                                kind="ExternalInput").ap()
    out_local = nc.dram_tensor("out_local", [R2, 64], F32,
                               kind="ExternalOutput").ap()

    hxA = nc.dram_tensor("hxA", [tA * P, ROW1], F16, kind="Internal").ap()
    hxB = nc.dram_tensor("hxB", [(NTILE - tA) * P, ROW1], F16,
                         kind="Internal").ap()
    hx2_src = nc.dram_tensor("hx2_src", [R2, ROW2], F16, kind="Internal").ap()
    hx2_all = nc.dram_tensor("hx2_all", [NCORES * R2, ROW2], F16,
                             kind="Internal", addr_space="Shared").ap()

    with tile.TileContext(nc) as tc:
        _emit(tc, nc, d, out_local, hxA, hxB, hx2_src, hx2_all,
              W, tA, tB, NTILE)
        _log("emit done (exiting TileContext = scheduling)...")
    _log("tile scheduling done; bacc compile...")
    nc.compile()
    _log("bacc compile done")
    return nc


def _emit(tc, nc, d, out_local, hxA, hxB, hx2_src, hx2_all, W, tA, tB, NTILE):
    AL = mybir.AluOpType
    ACT = mybir.ActivationFunctionType
    NS = W * SUBT

    import contextlib
    ctx = contextlib.ExitStack()
    with ctx:
        const = ctx.enter_context(tc.tile_pool(name="const", bufs=1))
        sb = ctx.enter_context(tc.tile_pool(name="sb", bufs=3))
        gpool = ctx.enter_context(tc.tile_pool(name="g", bufs=3))
        tpool = ctx.enter_context(tc.tile_pool(name="t", bufs=3))
        ohp = ctx.enter_context(tc.tile_pool(name="ohp", bufs=3))
        psw = ctx.enter_context(tc.tile_pool(name="psw", bufs=2, space="PSUM"))
        pse = ctx.enter_context(tc.tile_pool(name="pse", bufs=2, space="PSUM"))
        psm = ctx.enter_context(tc.tile_pool(name="psm", bufs=2, space="PSUM"))

        # ---- resident constants
        iota_t = const.tile([P, P], F16)
        nc.sync.dma_start(iota_t[:], d["iota"][:])
        ident_t = const.tile([P, P], F16)
        nc.sync.dma_start(ident_t[:], d["ident"][:])
        w1_t = const.tile([P, 136], F16)
        nc.sync.dma_start(w1_t[:], d["w1ext"][:])
        w2_t = const.tile([P, 68], F16)
        nc.sync.dma_start(w2_t[:], d["w2ext"][:])
        headb_t = const.tile([P, 64], F32)
        nc.sync.dma_start(headb_t[:], d["headb"][:])
        colv_t = const.tile([P, NS], F16)
        nc.sync.dma_start(colv_t[:], d["colv"][:])
        idx1_t = const.tile([P, 2 * W * 64], I16)
        nc.sync.dma_start(idx1_t[:], d["l1idx"][:])
        idx2_t = const.tile([P, 2 * W * 64], I16)
        nc.sync.dma_start(idx2_t[:], d["l2idx"][:])
        ed1_sb = const.tile([P, W * 4], F16)      # e_dst L1, window-local
        ed2_sb = const.tile([P, W], F16)          # e_dst L2

        # ================= P1: build hx tables (replicated) ================
        CH = 32                                   # node tiles per xT chunk
        nchunk = (NTILE + CH - 1) // CH
        stages = [const.tile([P, CH * ROW1], F16, name=f"stage{i}")
                  for i in range(2)]
        for st in stages:
            nc.vector.memset(st[:], 0.0)
        h2stages = [const.tile([P, ROW2], F16, name=f"h2st{i}")
                    for i in range(2)]
        for st in h2stages:
            nc.vector.memset(st[:], 0.0)
        for ch in range(nchunk):
            t0 = ch * CH
            t1 = min(t0 + CH, NTILE)
            xt_c = sb.tile([P, CH * P], F16, tag="xtc")
            nc.sync.dma_start(xt_c[:, :(t1 - t0) * P],
                              d["xT"][:, t0 * P:t1 * P])
            stage = stages[ch % 2]
            for t in range(t0, t1):
                ps = psm.tile([P, 136], F32, space="PSUM", tag="psmA")
                nc.tensor.matmul(ps[:], lhsT=xt_c[:, (t - t0) * P:(t - t0 + 1) * P],
                                 rhs=w1_t[:, 0:136], start=True, stop=True)
                nc.scalar.activation(
                    stage[:, (t - t0) * ROW1:(t - t0) * ROW1 + 132],
                    ps[:, 0:132], ACT.Copy)
            # write chunk rows to hxA/hxB
            for t in range(t0, t1):
                src = stage[:, (t - t0) * ROW1:(t - t0 + 1) * ROW1]
                if t < tA:
                    dst = hxA[t * P:(t + 1) * P, :]
                else:
                    dst = hxB[(t - tA) * P:(t - tA + 1) * P, :]
                nc.sync.dma_start(dst, src[:, 0:ROW1])

        # e_dst for own nodes, window-local order
        for w in range(W):
            xl = sb.tile([P, P], F16, tag="xloc")
            nc.sync.dma_start(xl[:], d["xlocT"][:, w * P:(w + 1) * P])
            ps = psm.tile([P, 8], F32, space="PSUM", tag="psmA")
            nc.tensor.matmul(ps[:, 0:4], lhsT=xl[:], rhs=w1_t[:, 132:136],
                             start=True, stop=True)
            nc.scalar.activation(ed1_sb[:, w * 4:(w + 1) * 4], ps[:, 0:4],
                                 ACT.Copy)

        # ================= edge-processing layers ==========================
        def layer(li):
            tabA = hxA if li == 1 else hx2_all[0:4 * W * P, :]
            tabB = hxB if li == 1 else hx2_all[4 * W * P:NCORES * W * P, :]
            rowlen = ROW1 if li == 1 else ROW2
            nheads = 4 if li == 1 else 1
            escol = 128 if li == 1 else 64
            idx_t = idx1_t if li == 1 else idx2_t
            ed_sb = ed1_sb if li == 1 else ed2_sb
            tcols = 128 + nheads if li == 1 else 64 + 1
            for w in range(W):
                # --- gathers: one call per side (1024 idx = 8 subtiles)
                gA = gpool.tile([P, 8 * rowlen], F16, tag=f"gA{li}")
                gB = gpool.tile([P, 8 * rowlen], F16, tag=f"gB{li}")
                for side, g in ((0, gA), (1, gB)):
                    k = w * 2 + side
                    nc.gpsimd.dma_gather(
                        out_ap=g[:].rearrange("p (b e) -> p b e", e=rowlen),
                        in_ap=(tabA if side == 0 else tabB)[:],
                        idxs_ap=idx_t[:, k * 64:(k + 1) * 64],
                        num_idxs=CALL, num_idxs_reg=CALL,
                        elem_size=rowlen, queue_num=k % NQ)

                # --- e_dst expansion + alpha + exp, window level
                ex_win = tpool.tile([P, SUBT * nheads], F16, tag=f"ex{li}")
                alpha = tpool.tile([P, SUBT * nheads], F16, tag=f"al{li}")
                for j in range(SUBT):
                    side = j % 2
                    jj = j // 2
                    g = gA if side == 0 else gB
                    oht = ohp.tile([P, P], F16, tag=f"oht{li}")
                    s = w * SUBT + j
                    nc.sync.dma_start(oht[:], d["ohT"][s * P:(s + 1) * P, :])
                    edp = pse.tile([P, nheads], F32, space="PSUM",
                                   tag="edp")
                    nc.tensor.matmul(
                        edp[:], lhsT=oht[:],
                        rhs=ed_sb[:, w * nheads:(w + 1) * nheads],
                        start=True, stop=True)
                    edc = tpool.tile([P, nheads], F16, tag="edc")
                    nc.scalar.activation(edc[:], edp[:, 0:nheads], ACT.Copy)
                    # alpha_j = es + ed
                    nc.vector.tensor_tensor(
                        out=alpha[:, j * nheads:(j + 1) * nheads],
                        in0=g[:, jj * rowlen + escol:
                             jj * rowlen + escol + nheads],
                        in1=edc[:], op=AL.add)
                # leaky: max(a, 0.2a); then exp
                t02 = tpool.tile([P, SUBT * nheads], F16, tag=f"t02{li}")
                nc.vector.tensor_scalar_mul(t02[:], alpha[:], NEG_SLOPE)
                nc.vector.tensor_tensor(out=alpha[:], in0=alpha[:],
                                        in1=t02[:], op=AL.max)
                nc.scalar.activation(ex_win[:], alpha[:], ACT.Exp)

                # --- build T tiles + one-hot + aggregation matmuls
                pw = psw.tile([P, tcols], F32, space="PSUM", tag="pw")
                for j in range(SUBT):
                    side = j % 2
                    jj = j // 2
                    g = gA if side == 0 else gB
                    tt = tpool.tile([P, tcols], F16, tag=f"tt{li}")
                    if li == 1:
                        nc.vector.tensor_tensor(
                            out=tt[:, 0:128].rearrange(
                                "p (h f) -> p h f", h=4),
                            in0=g[:, jj * rowlen:jj * rowlen + 128].rearrange(
                                "p (h f) -> p h f", h=4),
                            in1=ex_win[:, j * 4:(j + 1) * 4].rearrange(
                                "p (h o) -> p h o", h=4).to_broadcast(
                                    [P, 4, 32]),
                            op=AL.mult)
                        nc.vector.tensor_copy(tt[:, 128:132],
                                              ex_win[:, j * 4:(j + 1) * 4])
                    else:
                        nc.vector.tensor_tensor(
                            out=tt[:, 0:64],
                            in0=g[:, jj * rowlen:jj * rowlen + 64],
                            in1=ex_win[:, j:j + 1].to_broadcast([P, 64]),
                            op=AL.mult)
                        nc.vector.tensor_copy(tt[:, 64:65],
                                              ex_win[:, j:j + 1])
                    oh = ohp.tile([P, P], F16, tag=f"oh{li}")
                    nc.vector.tensor_tensor(
                        out=oh[:], in0=iota_t[:],
                        in1=colv_t[:, w * SUBT + j:w * SUBT + j + 1
                                   ].to_broadcast([P, P]),
                        op=AL.is_equal)
                    nc.tensor.matmul(pw[:], lhsT=oh[:], rhs=tt[:],
                                     start=(j == 0), stop=(j == SUBT - 1))

                # --- window epilogue
                if li == 1:
                    den = tpool.tile([P, 4], F32, tag="dn1")
                    nc.vector.tensor_scalar_add(den[:], pw[:, 128:132], 1e-30)
                    recip = tpool.tile([P, 4], F32, tag="rc1")
                    nc.vector.reciprocal(recip[:], den[:])
                    o1 = tpool.tile([P, P], F16, tag="o1")
                    nc.vector.tensor_tensor(
                        out=o1[:].rearrange("p (h f) -> p h f", h=4),
                        in0=pw[:, 0:128].rearrange("p (h f) -> p h f", h=4),
                        in1=recip[:].rearrange("p (h o) -> p h o",
                                               h=4).to_broadcast([P, 4, 32]),
                        op=AL.mult)
                    nc.vector.tensor_scalar_max(o1[:], o1[:], 0.0)  # relu
                    o1tp = psm.tile([P, P], F16, space="PSUM", tag="psmA")
                    nc.tensor.transpose(o1tp[:], o1[:], ident_t[:])
                    o1T = tpool.tile([P, P], F16, tag="o1T")
                    nc.scalar.activation(o1T[:], o1tp[:], ACT.Copy)
                    h2p = psm.tile([P, 68], F32, space="PSUM", tag="psmB")
                    nc.tensor.matmul(h2p[:, 0:66], lhsT=o1T[:],
                                     rhs=w2_t[:, 0:66], start=True, stop=True)
                    h2s = h2stages[w % 2]
                    nc.scalar.activation(h2s[:, 0:65], h2p[:, 0:65], ACT.Copy)
                    nc.scalar.activation(ed2_sb[:, w:w + 1], h2p[:, 65:66],
                                         ACT.Copy)
                    nc.sync.dma_start(hx2_src[w * P:(w + 1) * P, :],
                                      h2s[:, 0:ROW2])
                else:
                    den = tpool.tile([P, 1], F32, tag="dn2")
                    nc.vector.tensor_scalar_add(den[:], pw[:, 64:65], 1e-30)
                    recip = tpool.tile([P, 1], F32, tag="rc2")
                    nc.vector.reciprocal(recip[:], den[:])
                    o2 = tpool.tile([P, 64], F32, tag="o2")
                    nc.vector.tensor_tensor(
                        out=o2[:], in0=pw[:, 0:64],
                        in1=recip[:].to_broadcast([P, 64]), op=AL.mult)
                    nc.vector.tensor_tensor(out=o2[:], in0=o2[:],
                                            in1=headb_t[:], op=AL.add)
                    nc.sync.dma_start(out_local[w * P:(w + 1) * P, :], o2[:])

        layer(1)
        # ---- AllGather h2 tables
        nc.gpsimd.collective_compute(
            "AllGather", mybir.AluOpType.bypass,
            replica_groups=[list(range(NCORES))],
            ins=[hx2_src[:]], outs=[hx2_all[:]])
        layer(2)


# ================================================================ entry point
def kernel(x, edge_index, W1, attn1, W2, attn2, head_W, head_b):
    x = np.asarray(x, np.float32)
    edge_index = np.asarray(edge_index)
    W1 = np.asarray(W1, np.float32)
    attn1 = np.asarray(attn1, np.float32)
    W2 = np.asarray(W2, np.float32)
    attn2 = np.asarray(attn2, np.float32)
    head_W = np.asarray(head_W, np.float32)
    head_b = np.asarray(head_b, np.float32)

    _log("host prep...")
    pp = _host_prep(x, edge_index, W1, attn1, W2, attn2, head_W, head_b)
    _log(f"host prep done W={pp['W']} windows={len(pp['windows'])}")
    in_maps = []
    for c in range(NCORES):
        ci = _build_core_inputs(pp, c, x, W1, attn1, W2, attn2,
                                head_W, head_b)
        in_maps.append({
            "xT": ci["xT"], "xlocT": ci["xlocT"], "w1ext": ci["w1ext"],
            "w2ext": ci["w2ext"], "l1idx": ci["l1idx"], "l2idx": ci["l2idx"],
            "colv": ci["colv"], "ohT": ci["ohT"], "iota": ci["iota"],
            "ident": ci["ident"], "headb": ci["headb"],
        })
    _log("inputs built; building program...")
    nc = _build_program(pp["W"], pp["tA"], pp["tB"], pp["NTILE"])
    _log("program built+compiled; running on hw...")
    res = bass_utils.run_bass_kernel_spmd(nc, in_maps, list(range(NCORES)))
    _log("hw run done")
    kernel._last_result = res
    kernel._last_nc = nc
    kernel._last_in_maps = in_maps

    out = np.zeros((N_NODES, 64), np.float32)
    W = pp["W"]
    for c in range(NCORES):
        pc = pp["per_core"][c]
        loc = res.results[c]["out_local"]          # [W*128, 64]
        for wi in range(pc["nw"]):
            wa, wb = pp["windows"][pp["core_w"][c][wi]]
            out[wa:wb] = loc[wi * P:wi * P + (wb - wa)]
    return out
